# revision 1
# baseline (speedup 1.0000x reference)
"""Bicubic grid_sample (transpose-like warp) for Trainium2, 8 NeuronCores.

Strategy: shard output rows across cores (256 rows/core). The warp maps
output (i, j) -> input (y ~ j +- 21, x ~ i +- 21), so each core needs an
x-column slab of the image. On device, repack the slab into a patch table
where each 256B unit holds the full 4x4x8ch bicubic patch at (y0, x0)
(fp16, x-replicated). Per 128x128-pixel tile: compute exact floors /
cubic weights on DVE/ACT, bulk dma_gather one 256B patch per pixel,
then weight-multiply + tree-reduce on DVE.
"""
import os, sys, types
sys.path.insert(0, "/opt/trn_rl_repo")
import numpy as np

try:  # register NTFF profile hook so BASS_TRACE=1 can measure HW time
    import antenv
    if "antenv.axon_hooks" not in sys.modules:
        from trn_agent_boot.trn_boot import _ntff_profile_via_ctypes
        _h = _ntff_profile_via_ctypes("/opt/axon/libaxon_pjrt.so")
        _m = types.ModuleType("antenv.axon_hooks")
        _m.get_axon_ntff_profile_hook = lambda: _h
        _m.set_axon_ntff_profile_hook = lambda h: None
        sys.modules["antenv.axon_hooks"] = _m
        antenv.axon_hooks = _m
except Exception:
    pass

import concourse.bass as bass
import concourse.bacc as bacc
import concourse.mybir as mybir
import concourse.tile as tile
from concourse import library_config
from concourse.bass_utils import run_bass_kernel_spmd

F32 = mybir.dt.float32
F16 = mybir.dt.float16
I16 = mybir.dt.int16
I32 = mybir.dt.int32
OP = mybir.AluOpType

N_CORES = 8
H = W = 2048
C = 8
RPC = H // N_CORES          # output rows per core = 256
PAD = 24                    # y halo rows on each side
YS = H + 2 * PAD            # 2096 slab rows
XS = 308                    # slab cols: [I0-24, I0+284)
XT = 176                    # table cols per row-group
YT = YS + 16                # table rows incl. pad so in_ap window stays in-bounds
JW = 128                    # j-chunk width
NJT = W // JW               # 16 j-tiles
A = -0.75                   # bicubic constant

N_JTILES = NJT              # reduced for dev runs if needed


def build_nc():
    nc = bacc.Bacc("TRN2", target_bir_lowering=False, debug=False,
                   num_devices=N_CORES, num_swdge_queues=4)
    xs = nc.dram_tensor("xs", [C, YS + 4, XS], F32, kind="ExternalInput")
    gr = nc.dram_tensor("gr", [RPC, W, 2], F32, kind="ExternalInput")
    out = nc.dram_tensor("out", [C, RPC, W], F32, kind="ExternalOutput")

    with tile.TileContext(nc) as tc:
        nc.gpsimd.load_library(library_config.mlp)
        import contextlib
        with contextlib.ExitStack() as ctx:
            _build_body(ctx, tc, nc, xs, gr, out)
    nc.compile()
    return nc


def _build_body(ctx, tc, nc, xs, gr, out):
    tabpool = ctx.enter_context(tc.tile_pool(name="tab", bufs=1, space="DRAM"))
    bncpool = ctx.enter_context(tc.tile_pool(name="bnc", bufs=4, space="DRAM"))
    gridp = ctx.enter_context(tc.tile_pool(name="grid", bufs=2))
    wrk = ctx.enter_context(tc.tile_pool(name="wrk", bufs=2))
    gp = ctx.enter_context(tc.tile_pool(name="g", bufs=2))
    lp = ctx.enter_context(tc.tile_pool(name="l", bufs=1))
    outp = ctx.enter_context(tc.tile_pool(name="out", bufs=2))

    # two patch tables, one per row-group: [YT*XT units, 128 fp16]
    tabs = []
    for g in range(2):
        tabg = tabpool.tile([YT * XT, 128], F16, tag=f"tab{g}")
        tabs.append(tabg)

    # ---------------- phase 1: repack xs -> tables ----------------
    # table unit (y', xu) elems [s4, r4, c8] = xs[c, y'+r, xu + s + 128*g]
    import contextlib
    with contextlib.ExitStack() as p1ctx:
        repack = p1ctx.enter_context(tc.tile_pool(name="repack", bufs=2))
        tpool = p1ctx.enter_context(tc.tile_pool(name="tgp", bufs=1))
        YB = 124
        n_yb = (YS + YB - 1) // YB
        for yb in range(n_yb):
            y0 = yb * YB
            rows = min(YB, YS - y0)
            tgs = []
            for g in range(2):
                tgt = tpool.tile([128, 179 * 32], F16, tag=f"tg{g}")
                tgs.append(tgt)
            for r in range(4):
                ld = rows
                for c in range(C):
                    t = repack.tile([128, XS], F32, tag=f"xsb{c}")
                    nc.sync.dma_start(t[:ld, :], xs[c, y0 + r:y0 + r + ld, :])
                    for g in range(2):
                        dst = bass.AP(tgs[g].tensor, tgs[g].offset + r * 8 + c,
                                      [[tgs[g].ap[0][0], ld], [32, 179]])
                        src = bass.AP(t.tensor, t.offset + 128 * g,
                                      [[t.ap[0][0], ld], [1, 179]])
                        if (r * C + c) % 2 == 0:
                            nc.vector.tensor_copy(dst, src)
                        else:
                            nc.scalar.copy(dst, src)
            for g in range(2):
                # one DMA writes all 4 s-planes: dst unit = [s,r,c] contiguous
                src = bass.AP(tgs[g].tensor, tgs[g].offset,
                              [[tgs[g].ap[0][0], rows], [32, XT], [32, 4], [1, 32]])
                dst = bass.AP(tabs[g].tensor, tabs[g].offset + y0 * XT * 128,
                              [[XT * 128, rows], [128, XT], [1, 128]])
                nc.sync.dma_start(dst, src)

    # ---------------- phase 2: per-tile gather + combine ----------------
    NI = 128 * JW                     # 16384 idxs per tile
    for g in range(2):
        IG = g * 128                  # row-group base (local to core rows)
        for J in range(N_JTILES):
            jb = J * JW
            ybase = jb                # table row offset for this tile
            # grid tile [128 rows, JW cols, 2]
            gt = gridp.tile([128, JW * 2], F32, tag="gt")
            nc.sync.dma_start(
                gt[:],
                bass.AP(gr, (IG) * W * 2 + jb * 2,
                        [[W * 2, 128], [1, JW * 2]]))
            gx = bass.AP(gt.tensor, gt.offset, [gt.ap[0], [2, JW]])
            gy = bass.AP(gt.tensor, gt.offset + 1, [gt.ap[0], [2, JW]])

            # lx = gx*1024 + (1047.5 - IG); ly = gy*1024 + (1046.5 - ybase)
            lx = wrk.tile([128, JW], F32, tag="lx")
            ly = wrk.tile([128, JW], F32, tag="ly")
            nc.scalar.activation(lx[:], gx, mybir.ActivationFunctionType.Copy,
                                 bias=1047.5 - IG, scale=1024.0)
            nc.scalar.activation(ly[:], gy, mybir.ActivationFunctionType.Copy,
                                 bias=1046.5 - ybase, scale=1024.0)

            def floorpair(v, tag):
                vi = wrk.tile([128, JW], I32, tag=f"vi{tag}")
                nc.vector.tensor_copy(vi[:], v[:])
                vf = wrk.tile([128, JW], F32, tag=f"vf{tag}")
                nc.vector.tensor_copy(vf[:], vi[:])
                co = wrk.tile([128, JW], F32, tag=f"co{tag}")
                nc.vector.tensor_tensor(co[:], vf[:], v[:], op=OP.is_gt)
                nc.vector.tensor_tensor(vf[:], vf[:], co[:], op=OP.subtract)
                fr = wrk.tile([128, JW], F32, tag=f"fr{tag}")
                nc.vector.tensor_tensor(fr[:], v[:], vf[:], op=OP.subtract)
                return vf, fr

            fx, tx = floorpair(lx, "x")   # fx = floor(lx), tx frac
            fy, ty = floorpair(ly, "y")

            # idx = fy*176 + fx - 1
            idxf = wrk.tile([128, JW], F32, tag="idxf")
            nc.vector.scalar_tensor_tensor(idxf[:], fy[:], float(XT), fx[:],
                                           op0=OP.mult, op1=OP.add)
            nc.scalar.activation(idxf[:], idxf[:], mybir.ActivationFunctionType.Copy, bias=-1.0, scale=1.0)
            idx16 = wrk.tile([128, JW], I16, tag="idx16")
            nc.vector.tensor_copy(idx16[:], idxf[:])

            # cubic weights for both dirs: w0..w3 as [128, JW] each
            def cubic(t, tag):
                # w0 = ((A*(t+1) - 5A)*(t+1) + 8A)*(t+1) - 4A
                s0 = wrk.tile([128, JW], F32, tag=f"s0{tag}")
                nc.scalar.activation(s0[:], t[:], mybir.ActivationFunctionType.Copy, bias=1.0, scale=1.0)
                w0 = wrk.tile([128, JW], F32, tag=f"w0{tag}")
                nc.scalar.activation(w0[:], s0[:],
                                     mybir.ActivationFunctionType.Copy,
                                     bias=-5.0 * A, scale=A)
                nc.vector.tensor_tensor(w0[:], w0[:], s0[:], op=OP.mult)
                nc.scalar.activation(w0[:], w0[:], mybir.ActivationFunctionType.Copy, bias=8.0 * A, scale=1.0)
                nc.vector.tensor_tensor(w0[:], w0[:], s0[:], op=OP.mult)
                nc.scalar.activation(w0[:], w0[:], mybir.ActivationFunctionType.Copy, bias=-4.0 * A, scale=1.0)
                # w1 = ((A+2)*t - (A+3))*t*t + 1
                w1 = wrk.tile([128, JW], F32, tag=f"w1{tag}")
                nc.scalar.activation(w1[:], t[:],
                                     mybir.ActivationFunctionType.Copy,
                                     bias=-(A + 3.0), scale=A + 2.0)
                t2 = wrk.tile([128, JW], F32, tag=f"t2{tag}")
                nc.vector.tensor_tensor(t2[:], t[:], t[:], op=OP.mult)
                nc.vector.tensor_tensor(w1[:], w1[:], t2[:], op=OP.mult)
                nc.scalar.activation(w1[:], w1[:], mybir.ActivationFunctionType.Copy, bias=1.0, scale=1.0)
                # u = 1 - t ; w2 = ((A+2)*u - (A+3))*u*u + 1
                u = wrk.tile([128, JW], F32, tag=f"u{tag}")
                nc.scalar.activation(u[:], t[:],
                                     mybir.ActivationFunctionType.Copy,
                                     bias=1.0, scale=-1.0)
                w2 = wrk.tile([128, JW], F32, tag=f"w2{tag}")
                nc.scalar.activation(w2[:], u[:],
                                     mybir.ActivationFunctionType.Copy,
                                     bias=-(A + 3.0), scale=A + 2.0)
                u2 = wrk.tile([128, JW], F32, tag=f"u2{tag}")
                nc.vector.tensor_tensor(u2[:], u[:], u[:], op=OP.mult)
                nc.vector.tensor_tensor(w2[:], w2[:], u2[:], op=OP.mult)
                nc.scalar.activation(w2[:], w2[:], mybir.ActivationFunctionType.Copy, bias=1.0, scale=1.0)
                # w3 = 1 - w0 - w1 - w2
                w3 = wrk.tile([128, JW], F32, tag=f"w3{tag}")
                nc.vector.tensor_tensor(w3[:], w0[:], w1[:], op=OP.add)
                nc.vector.tensor_tensor(w3[:], w3[:], w2[:], op=OP.add)
                nc.scalar.activation(w3[:], w3[:],
                                     mybir.ActivationFunctionType.Copy,
                                     bias=1.0, scale=-1.0)
                return w0, w1, w2, w3

            wx = cubic(tx, "x")
            wy = cubic(ty, "y")

            # pack wx into [128, JW*4] (s-minor), then outer product with wy
            wxp = wrk.tile([128, JW * 4], F32, tag="wxp")
            for s in range(4):
                dst = bass.AP(wxp.tensor, wxp.offset + s, [wxp.ap[0], [4, JW]])
                nc.scalar.copy(dst, wx[s][:])
            wp = wrk.tile([128, JW * 16], F16, tag="wp")
            for r in range(4):
                # wp[.., jj, s, r] = wxp[jj, s] * wy_r[jj]
                dst = bass.AP(wp.tensor, wp.offset + r,
                              [wp.ap[0], [16, JW], [4, 4]])
                src0 = bass.AP(wxp.tensor, wxp.offset,
                               [wxp.ap[0], [4, JW], [1, 4]])
                src1 = bass.AP(wy[r].tensor, wy[r].offset,
                               [wy[r].ap[0], [1, JW], [0, 4]])
                nc.vector.tensor_tensor(dst, src0, src1, op=OP.mult)

            # bounce idx to DRAM, read back wrapped+replicated
            bnc = bncpool.tile([128 * JW], I16, tag="bnc")
            nc.sync.dma_start(
                bass.AP(bnc.tensor, bnc.offset, [[JW, 128], [1, JW]]),
                idx16[:])
            idxw = wrk.tile([128, JW * 8], I16, tag="idxw")
            for k in range(8):
                nc.sync.dma_start(
                    bass.AP(idxw.tensor, idxw.offset + k,
                            [[idxw.ap[0][0], 16], [8, JW], [1, 1]]),
                    bass.AP(bnc.tensor, bnc.offset + k * 16 * JW,
                            [[JW, 16], [1, JW], [1, 1]]))
            for rep in range(1, 8):
                nc.sync.dma_start(
                    bass.AP(idxw.tensor, idxw.offset + 16 * rep * idxw.ap[0][0],
                            [[idxw.ap[0][0], 16], [1, JW * 8]]),
                    bass.AP(idxw.tensor, idxw.offset,
                            [[idxw.ap[0][0], 16], [1, JW * 8]]))

            # bulk gather: 16384 patches of 256B
            G = gp.tile([128, JW, 128], F16, tag="G")
            in_ap = bass.AP(tabs[g].tensor,
                            tabs[g].offset + ybase * XT * 128,
                            [[128, 186 * XT], [1, 128]])
            NSUB = 4096
            for m in range(NI // NSUB):
                nc.gpsimd.dma_gather(
                    out_ap=G[:, m * (NSUB // 128):(m + 1) * (NSUB // 128), :],
                    in_ap=in_ap,
                    idxs_ap=idxw[:, m * (NSUB // 16):(m + 1) * (NSUB // 16)],
                    num_idxs=NSUB,
                    num_idxs_reg=NSUB,
                    elem_size=128,
                    elem_step=128,
                    single_packet=False,
                    queue_num=(g * N_JTILES * 4 + J * 4 + m) % 4,
                )

            if os.environ.get("KDBG") and g == 0 and J == 0:
                dbgG = nc.dram_tensor("dbgG", [128, JW * 128], F16, kind="ExternalOutput")
                nc.sync.dma_start(bass.AP(dbgG, 0, [[JW * 128, 128], [1, JW * 128]]),
                                  bass.AP(G.tensor, G.offset, [G.ap[0], [1, JW * 128]]))
                dbgW = nc.dram_tensor("dbgW", [128, JW * 16], F16, kind="ExternalOutput")
                nc.sync.dma_start(bass.AP(dbgW, 0, [[JW * 16, 128], [1, JW * 16]]), wp[:])
                dbgI = nc.dram_tensor("dbgI", [128, JW * 8], I16, kind="ExternalOutput")
                nc.sync.dma_start(bass.AP(dbgI, 0, [[JW * 8, 128], [1, JW * 8]]), idxw[:])
            # combine: P = G * wp (bcast over c) in-place, tree-reduce, out f32
            src1 = bass.AP(wp.tensor, wp.offset,
                           [wp.ap[0], [16, JW], [4, 4], [1, 4], [0, 8]])
            src0 = bass.AP(G.tensor, G.offset,
                           [G.ap[0], [128, JW], [32, 4], [8, 4], [1, 8]])
            nc.vector.tensor_tensor(src0, src0, src1, op=OP.mult)
            P = G

            # reduce over s (stride 32): 4 -> 2 -> 1, then r (stride 8)
            def halve(buf, npx, stride, n, tag):
                # adds pairs along the dim with given stride/count n -> n/2
                o = lp.tile([128, JW * stride * (n // 2)], F16, tag=tag)
                i0 = bass.AP(buf.tensor, buf.offset,
                             [buf.ap[0], [stride * n, npx], [stride * 2, n // 2], [1, stride]])
                i1 = bass.AP(buf.tensor, buf.offset + stride,
                             [buf.ap[0], [stride * n, npx], [stride * 2, n // 2], [1, stride]])
                od = bass.AP(o.tensor, o.offset,
                             [o.ap[0], [stride * (n // 2), npx], [stride, n // 2], [1, stride]])
                nc.vector.tensor_tensor(od, i0, i1, op=OP.add)
                return o

            L1 = halve(P, JW, 32, 4, "L1")     # sum s pairs -> [jj, 2, r, c](64)
            L2 = halve(L1, JW, 32, 2, "L2")    # -> [jj, r4, c8](32)
            L3 = halve(L2, JW, 8, 4, "L3")     # sum r pairs -> [jj, 2, c](16)
            # final level: write f32 transposed to (c, jj)
            of = outp.tile([128, 8 * JW], F32, tag="of")
            i0 = bass.AP(L3.tensor, L3.offset, [L3.ap[0], [16, JW], [1, 8]])
            i1 = bass.AP(L3.tensor, L3.offset + 8, [L3.ap[0], [16, JW], [1, 8]])
            od = bass.AP(of.tensor, of.offset, [of.ap[0], [1, JW], [JW, 8]])
            nc.vector.tensor_tensor(od, i0, i1, op=OP.add)

            # write out[c, IG+p, jb:jb+JW]
            dsto = bass.AP(out, IG * W + jb,
                           [[W, 128], [RPC * W, 8], [1, JW]])
            nc.sync.dma_start(dsto, of[:])


_NC_CACHE = None


def kernel(x: np.ndarray, grid: np.ndarray) -> np.ndarray:
    global _NC_CACHE
    if _NC_CACHE is None:
        _NC_CACHE = build_nc()
    nc = _NC_CACHE

    x0 = np.ascontiguousarray(x[0], dtype=np.float32)        # [C, H, W]
    g0 = np.ascontiguousarray(grid[0], dtype=np.float32)     # [H, W, 2]

    in_maps = []
    for k in range(N_CORES):
        I0 = k * RPC
        xsl = np.zeros((C, YS + 4, XS), dtype=np.float32)
        c0 = I0 - PAD
        lo, hi = max(0, c0), min(W, c0 + XS)
        xsl[:, PAD:PAD + H, lo - c0:hi - c0] = x0[:, :, lo:hi]
        grc = np.ascontiguousarray(g0[I0:I0 + RPC]).copy()
        grc[..., 0] -= I0 / 1024.0   # fold per-core x-base into gx
        in_maps.append({"xs": xsl, "gr": grc})

    res = run_bass_kernel_spmd(nc, in_maps, core_ids=list(range(N_CORES)),
                               trace=False)
    global _LAST_EXEC_NS
    _LAST_EXEC_NS = res.exec_time_ns
    out = np.empty((1, C, H, W), dtype=np.float32)
    for k in range(N_CORES):
        out[0, :, k * RPC:(k + 1) * RPC, :] = res.results[k]["out"]
    return out



# revision 6
# speedup vs baseline: 1.1476x; 1.1476x over previous
"""Bicubic grid_sample (transpose-like warp) for Trainium2, 8 NeuronCores.

Strategy: shard output rows across cores (256 rows/core). The warp maps
output (i, j) -> input (y ~ j +- 22, x ~ i +- 22), so each core needs an
x-column slab of the image. On device, repack the slab into a patch table
where each 256B token holds the full 4x4x8ch bicubic patch at (y0, x0)
(fp16, [s4 r4 c8] layout). Tokens are assembled fully in SBUF (DVE/ACT do
the 16x replication) so the table-write DMA moves ~78KB contiguous bursts
per partition; the baseline version let the DMA replicate via 64B reads
and ran at ~48GB/s. Phase 2: per 128x64-pixel subtile, compute floors /
cubic weights on DVE/ACT at block granularity, bulk dma_gather one 256B
patch per pixel, then weight-multiply + tree-reduce on DVE.
"""
import os, sys, types
sys.path.insert(0, "/opt/trn_rl_repo")
import numpy as np

try:  # register NTFF profile hook so BASS_TRACE=1 can measure HW time
    import antenv
    if "antenv.axon_hooks" not in sys.modules:
        from trn_agent_boot.trn_boot import _ntff_profile_via_ctypes
        _h = _ntff_profile_via_ctypes("/opt/axon/libaxon_pjrt.so")
        _m = types.ModuleType("antenv.axon_hooks")
        _m.get_axon_ntff_profile_hook = lambda: _h
        _m.set_axon_ntff_profile_hook = lambda h: None
        sys.modules["antenv.axon_hooks"] = _m
        antenv.axon_hooks = _m
except Exception:
    pass

import concourse.bass as bass
import concourse.bacc as bacc
import concourse.mybir as mybir
import concourse.tile as tile
from concourse import library_config
from concourse.bass_utils import run_bass_kernel_spmd

F32 = mybir.dt.float32
F16 = mybir.dt.float16
I16 = mybir.dt.int16
I32 = mybir.dt.int32
OP = mybir.AluOpType
ACTF = mybir.ActivationFunctionType

N_CORES = 8
H = W = 2048
C = 8
RPC = H // N_CORES          # output rows per core = 256
PAD = 24                    # y halo rows on each side
YS = H + 2 * PAD            # 2096 slab rows
XS = 308                    # slab cols: [I0-24, I0+284)
XT = 304                    # table x positions (token x0-1 in [0, 304))
TABR = YS + 4               # table rows
JB = 256                    # j-block width (weights computed per block)
JW = 64                     # j-subtile width (one gather per subtile)
A = -0.75                   # bicubic constant


def build_nc():
    nc = bacc.Bacc("TRN2", target_bir_lowering=False, debug=False,
                   num_devices=N_CORES, num_swdge_queues=4)
    xs = nc.dram_tensor("xs", [C, YS + 4, XS], F32, kind="ExternalInput")
    gr = nc.dram_tensor("gr", [RPC, W, 2], F32, kind="ExternalInput")
    out = nc.dram_tensor("out", [C, RPC, W], F32, kind="ExternalOutput")

    with tile.TileContext(nc) as tc:
        nc.gpsimd.load_library(library_config.mlp)
        import contextlib
        with contextlib.ExitStack() as ctx:
            _build_body(ctx, tc, nc, xs, gr, out)
    nc.compile()
    return nc


def _build_body(ctx, tc, nc, xs, gr, out):
    tabpool = ctx.enter_context(tc.tile_pool(name="tab", bufs=1, space="DRAM"))
    # patch table: token (y, x) = [s4, r4, c8] f16 = xs[c, y+r, x+s]
    tab = tabpool.tile([TABR * XT, 128], F16, tag="tab")

    # ---------------- phase 1: repack xs -> table ----------------
    import contextlib
    with contextlib.ExitStack() as p1:
        ldp = p1.enter_context(tc.tile_pool(name="ld", bufs=2))
        cvp = p1.enter_context(tc.tile_pool(name="cv", bufs=2))
        tgp = p1.enter_context(tc.tile_pool(name="tg", bufs=1))
        YBK = 128
        n_yb = (YS + YBK - 1) // YBK
        for yb in range(n_yb):
            y0 = yb * YBK
            rows = min(YBK, YS - y0)
            tg = tgp.tile([128, XT * 128], F16, tag="tg")
            for r in range(4):
                # load slab rows y0+r .. y0+r+rows as [y, (c, x)] f32
                t = ldp.tile([128, C * XS], F32, tag=f"ld{r}")
                src = bass.AP(xs, (y0 + r) * XS,
                              [[XS, rows], [(YS + 4) * XS, C], [1, XS]])
                nc.sync.dma_start(
                    bass.AP(t.tensor, t.offset,
                            [[t.ap[0][0], rows], [XS, C], [1, XS]]), src)
                # convert-transpose to f16 [y, (x, c)]
                f = cvp.tile([128, XS * C], F16, tag=f"cv{r}")
                cdst = bass.AP(f.tensor, f.offset,
                               [[f.ap[0][0], rows], [1, C], [C, XS]])
                csrc = bass.AP(t.tensor, t.offset,
                               [[t.ap[0][0], rows], [XS, C], [1, XS]])
                if r % 2 == 0:
                    nc.vector.tensor_copy(cdst, csrc)
                else:
                    nc.scalar.copy(cdst, csrc)
                # scatter into tokens: tg[y, x*128 + s*32 + r*8 + c]
                #   <- f[y, (x+s)*8 + c]   (overlapping reads over s)
                sdst = bass.AP(tg.tensor, tg.offset + r * 8,
                               [[tg.ap[0][0], rows], [128, XT], [32, 4], [1, 8]])
                ssrc = bass.AP(f.tensor, f.offset,
                               [[f.ap[0][0], rows], [C, XT], [C, 4], [1, 8]])
                if r % 2 == 0:
                    nc.scalar.copy(sdst, ssrc)
                else:
                    nc.vector.tensor_copy(sdst, ssrc)
            # one big-burst DMA: row y -> tab[(y0+y)*XT .. +XT) tokens
            # (split the 77824B/partition run into <64KB descriptor halves)
            nc.sync.dma_start(
                bass.AP(tab.tensor, tab.offset + y0 * XT * 128,
                        [[XT * 128, rows], [XT * 64, 2], [1, XT * 64]]),
                bass.AP(tg.tensor, tg.offset,
                        [[tg.ap[0][0], rows], [XT * 64, 2], [1, XT * 64]]))

    # ---------------- phase 2: per-subtile gather + combine ----------------
    gridp = ctx.enter_context(tc.tile_pool(name="grid", bufs=2))
    wrk = ctx.enter_context(tc.tile_pool(name="wrk", bufs=2))
    wrks = ctx.enter_context(tc.tile_pool(name="wrks", bufs=2))
    bncpool = ctx.enter_context(tc.tile_pool(name="bnc", bufs=4, space="DRAM"))
    gp = ctx.enter_context(tc.tile_pool(name="g", bufs=2))
    lp = ctx.enter_context(tc.tile_pool(name="l", bufs=1))
    outp = ctx.enter_context(tc.tile_pool(name="out", bufs=2))

    NSUB = JB // JW                   # subtiles per block
    qn = 0
    for g in range(2):                # i-halves: rows IG..IG+127
        IG = g * 128
        for jblk in range(W // JB):
            jb = jblk * JB
            # grid tile [128 rows, JB cols, 2]
            gt = gridp.tile([128, JB * 2], F32, tag="gt")
            nc.sync.dma_start(
                gt[:],
                bass.AP(gr, IG * W * 2 + jb * 2, [[W * 2, 128], [1, JB * 2]]))
            gx = bass.AP(gt.tensor, gt.offset, [gt.ap[0], [2, JB]])
            gy = bass.AP(gt.tensor, gt.offset + 1, [gt.ap[0], [2, JB]])

            # lx = gx*1024 + 1046.5 (slab x of leftmost tap x0-1)
            # ly = gy*1024 + 1045.5 - jb (block-local y of top tap y0-1)
            lx = wrk.tile([128, JB], F32, tag="lx")
            ly = wrk.tile([128, JB], F32, tag="ly")
            nc.scalar.activation(lx[:], gx, ACTF.Copy, bias=1046.5, scale=1024.0)
            nc.scalar.activation(ly[:], gy, ACTF.Copy,
                                 bias=1044.5 - jb, scale=1024.0)

            def floorpair(v, tag):
                vi = wrk.tile([128, JB], I32, tag=f"vi{tag}")
                nc.vector.tensor_copy(vi[:], v[:])
                vf = wrk.tile([128, JB], F32, tag=f"vf{tag}")
                nc.vector.tensor_copy(vf[:], vi[:])
                co = wrk.tile([128, JB], F32, tag=f"co{tag}")
                nc.vector.tensor_tensor(co[:], vf[:], v[:], op=OP.is_gt)
                nc.vector.tensor_tensor(vf[:], vf[:], co[:], op=OP.subtract)
                fr = wrk.tile([128, JB], F32, tag=f"fr{tag}")
                nc.vector.tensor_tensor(fr[:], v[:], vf[:], op=OP.subtract)
                return vf, fr

            fx, tx = floorpair(lx, "x")   # fx = x0-1 (slab), tx frac
            fy, ty = floorpair(ly, "y")   # fy = y0-1 (block-local), ty frac

            # idxf = fy*XT + fx  (block-local; subtile rebases by -64*t*XT)
            idxf = wrk.tile([128, JB], F32, tag="idxf")
            nc.vector.scalar_tensor_tensor(idxf[:], fy[:], float(XT), fx[:],
                                           op0=OP.mult, op1=OP.add)

            # cubic weights for both dirs
            def cubic(t, tag):
                s0 = wrk.tile([128, JB], F32, tag=f"s0{tag}")
                nc.scalar.activation(s0[:], t[:], ACTF.Copy, bias=1.0, scale=1.0)
                w0 = wrk.tile([128, JB], F32, tag=f"w0{tag}")
                nc.scalar.activation(w0[:], s0[:], ACTF.Copy,
                                     bias=-5.0 * A, scale=A)
                nc.vector.tensor_tensor(w0[:], w0[:], s0[:], op=OP.mult)
                nc.scalar.activation(w0[:], w0[:], ACTF.Copy, bias=8.0 * A, scale=1.0)
                nc.vector.tensor_tensor(w0[:], w0[:], s0[:], op=OP.mult)
                nc.scalar.activation(w0[:], w0[:], ACTF.Copy, bias=-4.0 * A, scale=1.0)
                w1 = wrk.tile([128, JB], F32, tag=f"w1{tag}")
                nc.scalar.activation(w1[:], t[:], ACTF.Copy,
                                     bias=-(A + 3.0), scale=A + 2.0)
                t2 = wrk.tile([128, JB], F32, tag=f"t2{tag}")
                nc.vector.tensor_tensor(t2[:], t[:], t[:], op=OP.mult)
                nc.vector.tensor_tensor(w1[:], w1[:], t2[:], op=OP.mult)
                nc.scalar.activation(w1[:], w1[:], ACTF.Copy, bias=1.0, scale=1.0)
                u = wrk.tile([128, JB], F32, tag=f"u{tag}")
                nc.scalar.activation(u[:], t[:], ACTF.Copy, bias=1.0, scale=-1.0)
                w2 = wrk.tile([128, JB], F32, tag=f"w2{tag}")
                nc.scalar.activation(w2[:], u[:], ACTF.Copy,
                                     bias=-(A + 3.0), scale=A + 2.0)
                u2 = wrk.tile([128, JB], F32, tag=f"u2{tag}")
                nc.vector.tensor_tensor(u2[:], u[:], u[:], op=OP.mult)
                nc.vector.tensor_tensor(w2[:], w2[:], u2[:], op=OP.mult)
                nc.scalar.activation(w2[:], w2[:], ACTF.Copy, bias=1.0, scale=1.0)
                w3 = wrk.tile([128, JB], F32, tag=f"w3{tag}")
                nc.vector.tensor_tensor(w3[:], w0[:], w1[:], op=OP.add)
                nc.vector.tensor_tensor(w3[:], w3[:], w2[:], op=OP.add)
                nc.scalar.activation(w3[:], w3[:], ACTF.Copy, bias=1.0, scale=-1.0)
                return w0, w1, w2, w3

            wx = cubic(tx, "x")
            wy = cubic(ty, "y")

            # pack wx into [128, JB*4] (s-minor), then outer product with wy
            wxp = wrk.tile([128, JB * 4], F32, tag="wxp")
            for s in range(4):
                dst = bass.AP(wxp.tensor, wxp.offset + s, [wxp.ap[0], [4, JB]])
                nc.scalar.copy(dst, wx[s][:])
            wp = wrk.tile([128, JB * 16], F16, tag="wp")
            for r in range(4):
                # wp[.., jj, s, r] = wxp[jj, s] * wy_r[jj]
                dst = bass.AP(wp.tensor, wp.offset + r,
                              [wp.ap[0], [16, JB], [4, 4]])
                src0 = bass.AP(wxp.tensor, wxp.offset,
                               [wxp.ap[0], [4, JB], [1, 4]])
                src1 = bass.AP(wy[r].tensor, wy[r].offset,
                               [wy[r].ap[0], [1, JB], [0, 4]])
                nc.vector.tensor_tensor(dst, src0, src1, op=OP.mult)

            for t in range(NSUB):
                jsub = jb + t * JW
                ybase = jsub + 2                # table row of fy_local = 0
                # idx16 = idxf[:, t*JW:(t+1)*JW] - t*JW*XT  (subtile rebase)
                idxs = wrks.tile([128, JW], F32, tag="idxs")
                islice = bass.AP(idxf.tensor, idxf.offset + t * JW,
                                 [idxf.ap[0], [1, JW]])
                nc.scalar.activation(idxs[:], islice, ACTF.Copy,
                                     bias=float(-t * JW * XT), scale=1.0)
                idx16 = wrks.tile([128, JW], I16, tag="idx16")
                nc.vector.tensor_copy(idx16[:], idxs[:])

                # bounce idx to DRAM, read back wrapped, replicate to 128
                bnc = bncpool.tile([128 * JW], I16, tag="bnc")
                nc.sync.dma_start(
                    bass.AP(bnc.tensor, bnc.offset, [[JW, 128], [1, JW]]),
                    idx16[:])
                idxw = wrks.tile([128, JW * 8], I16, tag="idxw")
                # idxs_ap[q, m] = idx of pixel n = m*16+q ; n = jj*128 + p
                #   -> q = p%16, m = jj*8 + p//16; bnc holds p-major [p, jj]
                nc.sync.dma_start(
                    bass.AP(idxw.tensor, idxw.offset,
                            [[idxw.ap[0][0], 16], [8, JW], [1, 8]]),
                    bass.AP(bnc.tensor, bnc.offset,
                            [[JW, 16], [1, JW], [16 * JW, 8]]))
                for k in range(3):
                    p = 16 << k
                    nc.sync.dma_start(
                        bass.AP(idxw.tensor,
                                idxw.offset + p * idxw.ap[0][0],
                                [[idxw.ap[0][0], p], [1, JW * 8]]),
                        bass.AP(idxw.tensor, idxw.offset,
                                [[idxw.ap[0][0], p], [1, JW * 8]]))

                # bulk gather: 8192 patches of 256B
                G = gp.tile([128, JW, 128], F16, tag="G")
                in_ap = bass.AP(tab.tensor,
                                tab.offset + ybase * XT * 128,
                                [[128, 107 * XT], [1, 128]])
                NI = 128 * JW
                nc.gpsimd.dma_gather(
                    out_ap=G[:, :, :],
                    in_ap=in_ap,
                    idxs_ap=idxw[:],
                    num_idxs=NI,
                    num_idxs_reg=NI,
                    elem_size=128,
                    elem_step=128,
                    single_packet=False,
                    queue_num=qn % 4,
                )
                qn += 1

                # combine: P = G * wp (bcast over c) in-place, tree-reduce
                src1 = bass.AP(wp.tensor, wp.offset + t * JW * 16,
                               [wp.ap[0], [16, JW], [4, 4], [1, 4], [0, 8]])
                src0 = bass.AP(G.tensor, G.offset,
                               [G.ap[0], [128, JW], [32, 4], [8, 4], [1, 8]])
                nc.vector.tensor_tensor(src0, src0, src1, op=OP.mult)
                P = G

                def halve(buf, npx, stride, n, tag):
                    o = lp.tile([128, JW * stride * (n // 2)], F16, tag=tag)
                    i0 = bass.AP(buf.tensor, buf.offset,
                                 [buf.ap[0], [stride * n, npx],
                                  [stride * 2, n // 2], [1, stride]])
                    i1 = bass.AP(buf.tensor, buf.offset + stride,
                                 [buf.ap[0], [stride * n, npx],
                                  [stride * 2, n // 2], [1, stride]])
                    od = bass.AP(o.tensor, o.offset,
                                 [o.ap[0], [stride * (n // 2), npx],
                                  [stride, n // 2], [1, stride]])
                    nc.vector.tensor_tensor(od, i0, i1, op=OP.add)
                    return o

                L1 = halve(P, JW, 32, 4, "L1")   # sum s pairs
                L2 = halve(L1, JW, 32, 2, "L2")
                L3 = halve(L2, JW, 8, 4, "L3")   # sum r pairs
                of = outp.tile([128, 8 * JW], F32, tag="of")
                i0 = bass.AP(L3.tensor, L3.offset, [L3.ap[0], [16, JW], [1, 8]])
                i1 = bass.AP(L3.tensor, L3.offset + 8, [L3.ap[0], [16, JW], [1, 8]])
                od = bass.AP(of.tensor, of.offset, [of.ap[0], [1, JW], [JW, 8]])
                nc.vector.tensor_tensor(od, i0, i1, op=OP.add)

                # write out[c, IG+p, jsub:jsub+JW]
                dsto = bass.AP(out, IG * W + jsub,
                               [[W, 128], [RPC * W, 8], [1, JW]])
                nc.sync.dma_start(dsto, of[:])


_NC_CACHE = None


def kernel(x: np.ndarray, grid: np.ndarray) -> np.ndarray:
    global _NC_CACHE
    if _NC_CACHE is None:
        _NC_CACHE = build_nc()
    nc = _NC_CACHE

    x0 = np.ascontiguousarray(x[0], dtype=np.float32)        # [C, H, W]
    g0 = np.ascontiguousarray(grid[0], dtype=np.float32)     # [H, W, 2]

    in_maps = []
    for k in range(N_CORES):
        I0 = k * RPC
        xsl = np.zeros((C, YS + 4, XS), dtype=np.float32)
        c0 = I0 - PAD
        lo, hi = max(0, c0), min(W, c0 + XS)
        xsl[:, PAD:PAD + H, lo - c0:hi - c0] = x0[:, :, lo:hi]
        grc = np.ascontiguousarray(g0[I0:I0 + RPC]).copy()
        grc[..., 0] -= I0 / 1024.0   # fold per-core x-base into gx
        in_maps.append({"xs": xsl, "gr": grc})

    res = run_bass_kernel_spmd(nc, in_maps, core_ids=list(range(N_CORES)),
                               trace=False)
    global _LAST_EXEC_NS, _LAST_RES
    _LAST_EXEC_NS = res.exec_time_ns
    _LAST_RES = res
    out = np.empty((1, C, H, W), dtype=np.float32)
    for k in range(N_CORES):
        out[0, :, k * RPC:(k + 1) * RPC, :] = res.results[k]["out"]
    return out


# revision 14
# speedup vs baseline: 1.8677x; 1.6274x over previous
"""Bicubic grid_sample (transpose-like warp) for Trainium2, 8 NeuronCores.

Strategy: shard output rows across cores (256 rows/core). The warp maps
output (i, j) -> input (y ~ j +- 22, x ~ i +- 22), so each core needs an
x-column slab of the image. On device, repack the slab into a patch table
where each 256B token holds the full 4x4x8ch bicubic patch at (y0, x0)
(fp16, [s4 r4 c8] layout). Tokens are assembled fully in SBUF (DVE/ACT do
the 16x replication) so the table-write DMA moves ~78KB contiguous bursts
per partition; the baseline version let the DMA replicate via 64B reads
and ran at ~48GB/s. Phase 2: per 128x64-pixel subtile, compute floors /
cubic weights on DVE/ACT at block granularity, bulk dma_gather one 256B
patch per pixel, then weight-multiply + tree-reduce on DVE.
"""
import os, sys, types
sys.path.insert(0, "/opt/trn_rl_repo")
import numpy as np

try:  # register NTFF profile hook so BASS_TRACE=1 can measure HW time
    import antenv
    if "antenv.axon_hooks" not in sys.modules:
        from trn_agent_boot.trn_boot import _ntff_profile_via_ctypes
        _h = _ntff_profile_via_ctypes("/opt/axon/libaxon_pjrt.so")
        _m = types.ModuleType("antenv.axon_hooks")
        _m.get_axon_ntff_profile_hook = lambda: _h
        _m.set_axon_ntff_profile_hook = lambda h: None
        sys.modules["antenv.axon_hooks"] = _m
        antenv.axon_hooks = _m
except Exception:
    pass

import concourse.bass as bass
import concourse.bacc as bacc
import concourse.mybir as mybir
import concourse.tile as tile
from concourse import library_config
from concourse.bass_utils import run_bass_kernel_spmd

F32 = mybir.dt.float32
F16 = mybir.dt.float16
I16 = mybir.dt.int16
I32 = mybir.dt.int32
OP = mybir.AluOpType
ACTF = mybir.ActivationFunctionType

N_CORES = 8
H = W = 2048
C = 8
RPC = H // N_CORES          # output rows per core = 256
PAD = 24                    # y halo rows on each side
YS = H + 2 * PAD            # 2096 slab rows
XS = 308                    # slab cols: [I0-24, I0+284)
XT = 304                    # table x positions (token x0-1 in [0, 304))
TABR = YS + 4               # table rows
JB = 256                    # j-block width (weights computed per block)
JW = 64                     # j-subtile width (one gather per subtile)
A = -0.75                   # bicubic constant


def build_nc():
    nc = bacc.Bacc("TRN2", target_bir_lowering=False, debug=False,
                   num_devices=N_CORES, num_swdge_queues=4)
    xs = nc.dram_tensor("xs", [YS + 4, XS, C], F32, kind="ExternalInput")
    gr = nc.dram_tensor("gr", [RPC, W, 2], F32, kind="ExternalInput")
    out = nc.dram_tensor("out", [C, RPC, W], F32, kind="ExternalOutput")

    with tile.TileContext(nc) as tc:
        nc.gpsimd.load_library(library_config.mlp)
        import contextlib
        with contextlib.ExitStack() as ctx:
            _build_body(ctx, tc, nc, xs, gr, out)
    nc.compile()
    return nc


def _build_body(ctx, tc, nc, xs, gr, out):
    tabpool = ctx.enter_context(tc.tile_pool(name="tab", bufs=1, space="DRAM"))
    # patch table: token (y, x) = [s4, r4, c8] f16 = xs[c, y+r, x+s]
    tab = tabpool.tile([TABR * XT, 128], F16, tag="tab")

    # ---------------- phase 1: repack xs -> table ----------------
    import contextlib
    with contextlib.ExitStack() as p1:
        ldp = p1.enter_context(tc.tile_pool(name="ld", bufs=2))
        tgp = p1.enter_context(tc.tile_pool(name="tg", bufs=1))
        YBK = 128
        n_yb = (YS + YBK - 1) // YBK
        for yb in range(n_yb):
            y0 = yb * YBK
            rows = min(YBK, YS - y0)
            tg = tgp.tile([128, XT * 128], F16, tag="tg")
            for r in range(4):
                # load slab rows y0+r .. y0+r+rows as [y, (x, c)] f32
                # (host ships xs as [Y, X, C], so rows are contiguous 9856B)
                t = ldp.tile([128, XS * C], F32, tag=f"ld{r}")
                src = bass.AP(xs, (y0 + r) * XS * C,
                              [[XS * C, rows], [1, XS * C]])
                nc.sync.dma_start(
                    bass.AP(t.tensor, t.offset,
                            [[t.ap[0][0], rows], [1, XS * C]]), src)
                # scatter into tokens: tg[y, x*128 + s*32 + r*8 + c]
                #   <- t[y, (x+s)*8 + c]  (contiguous f32 src, f16 dst)
                for s in range(4):
                    sdst = bass.AP(tg.tensor, tg.offset + s * 32 + r * 8,
                                   [[tg.ap[0][0], rows], [128, XT], [1, 8]])
                    ssrc = bass.AP(t.tensor, t.offset + s * C,
                                   [[t.ap[0][0], rows], [C, XT], [1, 8]])
                    if (r * 4 + s) % 2 == 0:
                        nc.scalar.copy(sdst, ssrc)
                    else:
                        nc.vector.tensor_copy(sdst, ssrc)
            # one big-burst DMA: row y -> tab[(y0+y)*XT .. +XT) tokens
            # (split the 77824B/partition run into <64KB descriptor halves)
            nc.sync.dma_start(
                bass.AP(tab.tensor, tab.offset + y0 * XT * 128,
                        [[XT * 128, rows], [XT * 64, 2], [1, XT * 64]]),
                bass.AP(tg.tensor, tg.offset,
                        [[tg.ap[0][0], rows], [XT * 64, 2], [1, XT * 64]]))

    # ---------------- phase 2: per-subtile gather + combine ----------------
    gridp = ctx.enter_context(tc.tile_pool(name="grid", bufs=2))
    wrk = ctx.enter_context(tc.tile_pool(name="wrk", bufs=2))
    wrks = ctx.enter_context(tc.tile_pool(name="wrks", bufs=2))
    bncpool = ctx.enter_context(tc.tile_pool(name="bnc", bufs=4, space="DRAM"))
    gp = ctx.enter_context(tc.tile_pool(name="g", bufs=2))
    lp = ctx.enter_context(tc.tile_pool(name="l", bufs=1))
    outp = ctx.enter_context(tc.tile_pool(name="out", bufs=2))

    NSUB = JB // JW                   # subtiles per block
    qn = 0
    for g in range(2):                # i-halves: rows IG..IG+127
        IG = g * 128
        for jblk in range(W // JB):
            jb = jblk * JB
            # grid tile [128 rows, JB cols, 2]
            gt = gridp.tile([128, JB * 2], F32, tag="gt")
            nc.sync.dma_start(
                gt[:],
                bass.AP(gr, IG * W * 2 + jb * 2, [[W * 2, 128], [1, JB * 2]]))
            gx = bass.AP(gt.tensor, gt.offset, [gt.ap[0], [2, JB]])
            gy = bass.AP(gt.tensor, gt.offset + 1, [gt.ap[0], [2, JB]])

            # lx = gx*1024 + 1046.5 (slab x of leftmost tap x0-1)
            # ly = gy*1024 + 1045.5 - jb (block-local y of top tap y0-1)
            lx = wrk.tile([128, JB], F32, tag="lx")
            ly = wrk.tile([128, JB], F32, tag="ly")
            nc.scalar.activation(lx[:], gx, ACTF.Copy, bias=1046.5, scale=1024.0)
            nc.scalar.activation(ly[:], gy, ACTF.Copy,
                                 bias=1044.5 - jb, scale=1024.0)

            def floorpair(v, tag):
                vi = wrk.tile([128, JB], I32, tag=f"vi{tag}")
                nc.vector.tensor_copy(vi[:], v[:])
                vf = wrk.tile([128, JB], F32, tag=f"vf{tag}")
                nc.vector.tensor_copy(vf[:], vi[:])
                co = wrk.tile([128, JB], F32, tag=f"co{tag}")
                nc.vector.tensor_tensor(co[:], vf[:], v[:], op=OP.is_gt)
                nc.vector.tensor_tensor(vf[:], vf[:], co[:], op=OP.subtract)
                fr = wrk.tile([128, JB], F32, tag=f"fr{tag}")
                nc.vector.tensor_tensor(fr[:], v[:], vf[:], op=OP.subtract)
                return vf, fr

            fx, tx = floorpair(lx, "x")   # fx = x0-1 (slab), tx frac
            fy, ty = floorpair(ly, "y")   # fy = y0-1 (block-local), ty frac

            # idxf = fy*XT + fx  (block-local; subtile rebases by -64*t*XT)
            idxf = wrk.tile([128, JB], F32, tag="idxf")
            nc.vector.scalar_tensor_tensor(idxf[:], fy[:], float(XT), fx[:],
                                           op0=OP.mult, op1=OP.add)

            # cubic weights for both dirs
            def cubic(t, tag):
                s0 = wrk.tile([128, JB], F32, tag=f"s0{tag}")
                nc.scalar.activation(s0[:], t[:], ACTF.Copy, bias=1.0, scale=1.0)
                w0 = wrk.tile([128, JB], F32, tag=f"w0{tag}")
                nc.scalar.activation(w0[:], s0[:], ACTF.Copy,
                                     bias=-5.0 * A, scale=A)
                nc.vector.tensor_tensor(w0[:], w0[:], s0[:], op=OP.mult)
                nc.scalar.activation(w0[:], w0[:], ACTF.Copy, bias=8.0 * A, scale=1.0)
                nc.vector.tensor_tensor(w0[:], w0[:], s0[:], op=OP.mult)
                nc.scalar.activation(w0[:], w0[:], ACTF.Copy, bias=-4.0 * A, scale=1.0)
                w1 = wrk.tile([128, JB], F32, tag=f"w1{tag}")
                nc.scalar.activation(w1[:], t[:], ACTF.Copy,
                                     bias=-(A + 3.0), scale=A + 2.0)
                t2 = wrk.tile([128, JB], F32, tag=f"t2{tag}")
                nc.vector.tensor_tensor(t2[:], t[:], t[:], op=OP.mult)
                nc.vector.tensor_tensor(w1[:], w1[:], t2[:], op=OP.mult)
                nc.scalar.activation(w1[:], w1[:], ACTF.Copy, bias=1.0, scale=1.0)
                u = wrk.tile([128, JB], F32, tag=f"u{tag}")
                nc.scalar.activation(u[:], t[:], ACTF.Copy, bias=1.0, scale=-1.0)
                w2 = wrk.tile([128, JB], F32, tag=f"w2{tag}")
                nc.scalar.activation(w2[:], u[:], ACTF.Copy,
                                     bias=-(A + 3.0), scale=A + 2.0)
                u2 = wrk.tile([128, JB], F32, tag=f"u2{tag}")
                nc.vector.tensor_tensor(u2[:], u[:], u[:], op=OP.mult)
                nc.vector.tensor_tensor(w2[:], w2[:], u2[:], op=OP.mult)
                nc.scalar.activation(w2[:], w2[:], ACTF.Copy, bias=1.0, scale=1.0)
                w3 = wrk.tile([128, JB], F32, tag=f"w3{tag}")
                nc.vector.tensor_tensor(w3[:], w0[:], w1[:], op=OP.add)
                nc.vector.tensor_tensor(w3[:], w3[:], w2[:], op=OP.add)
                nc.scalar.activation(w3[:], w3[:], ACTF.Copy, bias=1.0, scale=-1.0)
                return w0, w1, w2, w3

            wx = cubic(tx, "x")
            wy = cubic(ty, "y")

            # pack wx into [128, JB*4] (s-minor), then outer product with wy
            wxp = wrk.tile([128, JB * 4], F32, tag="wxp")
            for s in range(4):
                dst = bass.AP(wxp.tensor, wxp.offset + s, [wxp.ap[0], [4, JB]])
                nc.scalar.copy(dst, wx[s][:])
            wp = wrk.tile([128, JB * 16], F16, tag="wp")
            for r in range(4):
                # wp[.., jj, s, r] = wxp[jj, s] * wy_r[jj]
                dst = bass.AP(wp.tensor, wp.offset + r,
                              [wp.ap[0], [16, JB], [4, 4]])
                src0 = bass.AP(wxp.tensor, wxp.offset,
                               [wxp.ap[0], [4, JB], [1, 4]])
                src1 = bass.AP(wy[r].tensor, wy[r].offset,
                               [wy[r].ap[0], [1, JB], [0, 4]])
                nc.vector.tensor_tensor(dst, src0, src1, op=OP.mult)

            for t in range(NSUB):
                jsub = jb + t * JW
                ybase = jsub + 2                # table row of fy_local = 0
                # idx16 = idxf[:, t*JW:(t+1)*JW] - t*JW*XT  (subtile rebase)
                idxs = wrks.tile([128, JW], F32, tag="idxs")
                islice = bass.AP(idxf.tensor, idxf.offset + t * JW,
                                 [idxf.ap[0], [1, JW]])
                nc.scalar.activation(idxs[:], islice, ACTF.Copy,
                                     bias=float(-t * JW * XT), scale=1.0)
                idx16 = wrks.tile([128, JW], I16, tag="idx16")
                nc.vector.tensor_copy(idx16[:], idxs[:])

                # bounce idx to DRAM, read back wrapped, replicate to 128
                bnc = bncpool.tile([128 * JW], I16, tag="bnc")
                nc.sync.dma_start(
                    bass.AP(bnc.tensor, bnc.offset, [[JW, 128], [1, JW]]),
                    idx16[:])
                # readback in (h, jj)-major order: big contiguous bursts
                idxT = wrks.tile([128, JW * 8], I16, tag="idxT")
                nc.sync.dma_start(
                    bass.AP(idxT.tensor, idxT.offset,
                            [[idxT.ap[0][0], 16], [1, JW * 8]]),
                    bass.AP(bnc.tensor, bnc.offset,
                            [[JW, 16], [16 * JW, 8], [1, JW]]))
                # idxs_ap[q, m] = idx of pixel n = m*16+q ; n = jj*128 + p
                #   -> q = p%16, m = jj*8 + p//16 : reorder (h,jj)->(jj,h)
                idxw = wrks.tile([128, JW * 8], I16, tag="idxw")
                nc.vector.tensor_copy(
                    bass.AP(idxw.tensor, idxw.offset,
                            [[idxw.ap[0][0], 16], [8, JW], [1, 8]]),
                    bass.AP(idxT.tensor, idxT.offset,
                            [[idxT.ap[0][0], 16], [1, JW], [JW, 8]]))
                for k in range(3):
                    p = 16 << k
                    nc.sync.dma_start(
                        bass.AP(idxw.tensor,
                                idxw.offset + p * idxw.ap[0][0],
                                [[idxw.ap[0][0], p], [1, JW * 8]]),
                        bass.AP(idxw.tensor, idxw.offset,
                                [[idxw.ap[0][0], p], [1, JW * 8]]))

                # bulk gather: 8192 patches of 256B, in 4096-desc calls
                # (8192-desc calls measured ~77us of Pool time vs 2x10.5us)
                G = gp.tile([128, JW, 128], F16, tag="G")
                in_ap = bass.AP(tab.tensor,
                                tab.offset + ybase * XT * 128,
                                [[128, 107 * XT], [1, 128]])
                NI = 128 * JW
                NSP = 4096
                for m in range(NI // NSP):
                    nc.gpsimd.dma_gather(
                        out_ap=G[:, m * (NSP // 128):(m + 1) * (NSP // 128), :],
                        in_ap=in_ap,
                        idxs_ap=idxw[:, m * (NSP // 16):(m + 1) * (NSP // 16)],
                        num_idxs=NSP,
                        num_idxs_reg=NSP,
                        elem_size=128,
                        elem_step=128,
                        single_packet=False,
                        queue_num=qn % 4,
                    )
                    qn += 1

                # combine: P = G * wp (bcast over c), tree-reduce
                P = gp.tile([128, JW, 128], F16, tag="P")
                src1 = bass.AP(wp.tensor, wp.offset + t * JW * 16,
                               [wp.ap[0], [16, JW], [4, 4], [1, 4], [0, 8]])
                src0 = bass.AP(G.tensor, G.offset,
                               [G.ap[0], [128, JW], [32, 4], [8, 4], [1, 8]])
                pdst = bass.AP(P.tensor, P.offset,
                               [P.ap[0], [128, JW], [32, 4], [8, 4], [1, 8]])
                nc.vector.tensor_tensor(pdst, src0, src1, op=OP.mult)

                def halve(buf, npx, stride, n, tag):
                    o = lp.tile([128, JW * stride * (n // 2)], F16, tag=tag)
                    i0 = bass.AP(buf.tensor, buf.offset,
                                 [buf.ap[0], [stride * n, npx],
                                  [stride * 2, n // 2], [1, stride]])
                    i1 = bass.AP(buf.tensor, buf.offset + stride,
                                 [buf.ap[0], [stride * n, npx],
                                  [stride * 2, n // 2], [1, stride]])
                    od = bass.AP(o.tensor, o.offset,
                                 [o.ap[0], [stride * (n // 2), npx],
                                  [stride, n // 2], [1, stride]])
                    nc.vector.tensor_tensor(od, i0, i1, op=OP.add)
                    return o

                L1 = halve(P, JW, 32, 4, "L1")   # sum s pairs
                L2 = halve(L1, JW, 32, 2, "L2")
                L3 = halve(L2, JW, 8, 4, "L3")   # sum r pairs
                of = outp.tile([128, 8 * JW], F32, tag="of")
                i0 = bass.AP(L3.tensor, L3.offset, [L3.ap[0], [16, JW], [1, 8]])
                i1 = bass.AP(L3.tensor, L3.offset + 8, [L3.ap[0], [16, JW], [1, 8]])
                od = bass.AP(of.tensor, of.offset, [of.ap[0], [1, JW], [JW, 8]])
                nc.vector.tensor_tensor(od, i0, i1, op=OP.add)

                # write out[c, IG+p, jsub:jsub+JW]
                dsto = bass.AP(out, IG * W + jsub,
                               [[W, 128], [RPC * W, 8], [1, JW]])
                nc.sync.dma_start(dsto, of[:])


_NC_CACHE = None


def kernel(x: np.ndarray, grid: np.ndarray) -> np.ndarray:
    global _NC_CACHE
    if _NC_CACHE is None:
        _NC_CACHE = build_nc()
    nc = _NC_CACHE

    x0 = np.ascontiguousarray(x[0], dtype=np.float32)        # [C, H, W]
    g0 = np.ascontiguousarray(grid[0], dtype=np.float32)     # [H, W, 2]

    in_maps = []
    for k in range(N_CORES):
        I0 = k * RPC
        xsl = np.zeros((YS + 4, XS, C), dtype=np.float32)
        c0 = I0 - PAD
        lo, hi = max(0, c0), min(W, c0 + XS)
        xsl[PAD:PAD + H, lo - c0:hi - c0, :] = \
            x0[:, :, lo:hi].transpose(1, 2, 0)
        grc = np.ascontiguousarray(g0[I0:I0 + RPC]).copy()
        grc[..., 0] -= I0 / 1024.0   # fold per-core x-base into gx
        in_maps.append({"xs": xsl, "gr": grc})

    res = run_bass_kernel_spmd(nc, in_maps, core_ids=list(range(N_CORES)),
                               trace=False)
    global _LAST_EXEC_NS, _LAST_RES
    _LAST_EXEC_NS = res.exec_time_ns
    _LAST_RES = res
    out = np.empty((1, C, H, W), dtype=np.float32)
    for k in range(N_CORES):
        out[0, :, k * RPC:(k + 1) * RPC, :] = res.results[k]["out"]
    return out


# revision 21
# speedup vs baseline: 2.6441x; 1.4157x over previous
"""Bicubic grid_sample (transpose-like warp) for Trainium2, 8 NeuronCores.

Strategy: shard output rows across cores (256 rows/core). The warp maps
output (i, j) -> input (y ~ j +- 22, x ~ i +- 22), so each core needs an
x-column slab of the image. On device, repack the slab into a patch table
where each 256B token holds the full 4x4x8ch bicubic patch at (y0, x0)
(fp16, [s4 r4 c8] layout). Tokens are assembled fully in SBUF (DVE/ACT do
the 16x replication) so the table-write DMA moves ~78KB contiguous bursts
per partition; the baseline version let the DMA replicate via 64B reads
and ran at ~48GB/s. Phase 2: per 128x64-pixel subtile, compute floors /
cubic weights on DVE/ACT at block granularity, bulk dma_gather one 256B
patch per pixel, then weight-multiply + tree-reduce on DVE.
"""
import os, sys, types
sys.path.insert(0, "/opt/trn_rl_repo")
import numpy as np

try:  # register NTFF profile hook so BASS_TRACE=1 can measure HW time
    import antenv
    if "antenv.axon_hooks" not in sys.modules:
        from trn_agent_boot.trn_boot import _ntff_profile_via_ctypes
        _h = _ntff_profile_via_ctypes("/opt/axon/libaxon_pjrt.so")
        _m = types.ModuleType("antenv.axon_hooks")
        _m.get_axon_ntff_profile_hook = lambda: _h
        _m.set_axon_ntff_profile_hook = lambda h: None
        sys.modules["antenv.axon_hooks"] = _m
        antenv.axon_hooks = _m
except Exception:
    pass

import concourse.bass as bass
import concourse.bacc as bacc
import concourse.mybir as mybir
import concourse.tile as tile
from concourse import library_config
from concourse.bass_utils import run_bass_kernel_spmd

F32 = mybir.dt.float32
F16 = mybir.dt.float16
I16 = mybir.dt.int16
I32 = mybir.dt.int32
OP = mybir.AluOpType
ACTF = mybir.ActivationFunctionType

N_CORES = 8
H = W = 2048
C = 8
RPC = H // N_CORES          # output rows per core = 256
PAD = 24                    # y halo rows on each side
YS = H + 2 * PAD            # 2096 slab rows
XS = 308                    # slab cols: [I0-24, I0+284)
XT = 304                    # table x positions (token x0-1 in [0, 304))
TABR = YS + 4               # table rows
JB = 256                    # j-block width (weights computed per block)
JW = 64                     # j-subtile width (one gather per subtile)
A = -0.75                   # bicubic constant


def build_nc():
    nc = bacc.Bacc("TRN2", target_bir_lowering=False, debug=False,
                   num_devices=N_CORES, num_swdge_queues=4)
    xs = nc.dram_tensor("xs", [YS + 4, XS, C], F32, kind="ExternalInput")
    gr = nc.dram_tensor("gr", [RPC, W, 2], F32, kind="ExternalInput")
    out = nc.dram_tensor("out", [C, RPC, W], F32, kind="ExternalOutput")

    with tile.TileContext(nc) as tc:
        nc.gpsimd.load_library(library_config.mlp)
        import contextlib
        with contextlib.ExitStack() as ctx:
            _build_body(ctx, tc, nc, xs, gr, out)
    nc.compile()
    return nc


def _build_body(ctx, tc, nc, xs, gr, out):
    tabpool = ctx.enter_context(tc.tile_pool(name="tab", bufs=1, space="DRAM"))
    # patch table: token (y, x) = [s4, r4, c8] f16 = xs[c, y+r, x+s]
    tab = tabpool.tile([TABR * XT, 128], F16, tag="tab")

    # ---------------- phase 1: repack xs -> table ----------------
    import contextlib
    with contextlib.ExitStack() as p1:
        ldp = p1.enter_context(tc.tile_pool(name="ld", bufs=2))
        cvp = p1.enter_context(tc.tile_pool(name="cv", bufs=2))
        tgp = p1.enter_context(tc.tile_pool(name="tg", bufs=1))
        YBK = 128
        n_yb = (YS + YBK - 1) // YBK
        for yb in range(n_yb):
            y0 = yb * YBK
            rows = min(YBK, YS - y0)
            tg = tgp.tile([128, XT * 128], F16, tag="tg")
            for r in range(4):
                # load slab rows y0+r .. y0+r+rows as [y, (x, c)] f32
                # (host ships xs as [Y, X, C], so rows are contiguous 9856B)
                t = ldp.tile([128, XS * C], F32, tag=f"ld{r}")
                src = bass.AP(xs, (y0 + r) * XS * C,
                              [[XS * C, rows], [1, XS * C]])
                nc.sync.dma_start(
                    bass.AP(t.tensor, t.offset,
                            [[t.ap[0][0], rows], [1, XS * C]]), src)
                # contiguous f32 -> f16 convert (fast DVE mode)
                f = cvp.tile([128, XS * C], F16, tag=f"cv{r}")
                nc.vector.tensor_copy(
                    bass.AP(f.tensor, f.offset, [[f.ap[0][0], rows], [1, XS * C]]),
                    bass.AP(t.tensor, t.offset, [[t.ap[0][0], rows], [1, XS * C]]))
                # scatter into tokens: tg[y, x*128 + s*32 + r*8 + c]
                #   <- f[y, (x+s)*8 + c]  (f16 both sides)
                for s in range(4):
                    sdst = bass.AP(tg.tensor, tg.offset + s * 32 + r * 8,
                                   [[tg.ap[0][0], rows], [128, XT], [1, 8]])
                    ssrc = bass.AP(f.tensor, f.offset + s * C,
                                   [[f.ap[0][0], rows], [C, XT], [1, 8]])
                    if s % 2 == 0:
                        nc.scalar.copy(sdst, ssrc)
                    else:
                        nc.vector.tensor_copy(sdst, ssrc)
            # one big-burst DMA: row y -> tab[(y0+y)*XT .. +XT) tokens
            # (split the 77824B/partition run into <64KB descriptor halves)
            nc.sync.dma_start(
                bass.AP(tab.tensor, tab.offset + y0 * XT * 128,
                        [[XT * 128, rows], [XT * 64, 2], [1, XT * 64]]),
                bass.AP(tg.tensor, tg.offset,
                        [[tg.ap[0][0], rows], [XT * 64, 2], [1, XT * 64]]))

    # ---------------- phase 2: per-subtile gather + combine ----------------
    gridp = ctx.enter_context(tc.tile_pool(name="grid", bufs=2))
    wrk = ctx.enter_context(tc.tile_pool(name="wrk", bufs=1))
    wbp = ctx.enter_context(tc.tile_pool(name="wb", bufs=2))
    wrks = ctx.enter_context(tc.tile_pool(name="wrks", bufs=2))
    bncpool = ctx.enter_context(tc.tile_pool(name="bnc", bufs=4, space="DRAM"))
    gp = ctx.enter_context(tc.tile_pool(name="g", bufs=2))
    wxpool = ctx.enter_context(tc.tile_pool(name="wx", bufs=2))
    lp = ctx.enter_context(tc.tile_pool(name="l", bufs=2))
    outp = ctx.enter_context(tc.tile_pool(name="out", bufs=2))

    NSUB = JB // JW                   # subtiles per block
    qn = 0
    for g in range(2):                # i-halves: rows IG..IG+127
        IG = g * 128
        for jblk in range(W // JB):
            jb = jblk * JB
            # grid tile [128 rows, JB cols, 2]
            gt = gridp.tile([128, JB * 2], F32, tag="gt")
            nc.sync.dma_start(
                gt[:],
                bass.AP(gr, IG * W * 2 + jb * 2, [[W * 2, 128], [1, JB * 2]]))
            gx = bass.AP(gt.tensor, gt.offset, [gt.ap[0], [2, JB]])
            gy = bass.AP(gt.tensor, gt.offset + 1, [gt.ap[0], [2, JB]])

            # lx = gx*1024 + 1046.5 (slab x of leftmost tap x0-1)
            # ly = gy*1024 + 1045.5 - jb (block-local y of top tap y0-1)
            lx = wrk.tile([128, JB], F32, tag="lx")
            ly = wrk.tile([128, JB], F32, tag="ly")
            nc.scalar.activation(lx[:], gx, ACTF.Copy, bias=1046.5, scale=1024.0)
            nc.scalar.activation(ly[:], gy, ACTF.Copy,
                                 bias=1044.5 - jb, scale=1024.0)

            def floorpair(v, tag):
                vi = wrk.tile([128, JB], I32, tag=f"vi{tag}")
                nc.vector.tensor_copy(vi[:], v[:])
                vf = wrk.tile([128, JB], F32, tag=f"vf{tag}")
                nc.vector.tensor_copy(vf[:], vi[:])
                co = wrk.tile([128, JB], F32, tag=f"co{tag}")
                nc.vector.tensor_tensor(co[:], vf[:], v[:], op=OP.is_gt)
                nc.vector.tensor_tensor(vf[:], vf[:], co[:], op=OP.subtract)
                fr = wrk.tile([128, JB], F32, tag=f"fr{tag}")
                nc.vector.tensor_tensor(fr[:], v[:], vf[:], op=OP.subtract)
                return vf, fr

            fx, tx = floorpair(lx, "x")   # fx = x0-1 (slab), tx frac
            fy, ty = floorpair(ly, "y")   # fy = y0-1 (block-local), ty frac

            # idxf = fy*XT + fx  (block-local; subtile rebases by -64*t*XT)
            idxf = wrk.tile([128, JB], F32, tag="idxf")
            nc.vector.scalar_tensor_tensor(idxf[:], fy[:], float(XT), fx[:],
                                           op0=OP.mult, op1=OP.add)

            # cubic weights for both dirs
            def cubic(t, tag):
                s0 = wrk.tile([128, JB], F32, tag=f"s0{tag}")
                nc.scalar.activation(s0[:], t[:], ACTF.Copy, bias=1.0, scale=1.0)
                w0 = wrk.tile([128, JB], F32, tag=f"w0{tag}")
                nc.scalar.activation(w0[:], s0[:], ACTF.Copy,
                                     bias=-5.0 * A, scale=A)
                nc.vector.tensor_tensor(w0[:], w0[:], s0[:], op=OP.mult)
                nc.scalar.activation(w0[:], w0[:], ACTF.Copy, bias=8.0 * A, scale=1.0)
                nc.vector.tensor_tensor(w0[:], w0[:], s0[:], op=OP.mult)
                nc.scalar.activation(w0[:], w0[:], ACTF.Copy, bias=-4.0 * A, scale=1.0)
                w1 = wrk.tile([128, JB], F32, tag=f"w1{tag}")
                nc.scalar.activation(w1[:], t[:], ACTF.Copy,
                                     bias=-(A + 3.0), scale=A + 2.0)
                t2 = wrk.tile([128, JB], F32, tag=f"t2{tag}")
                nc.vector.tensor_tensor(t2[:], t[:], t[:], op=OP.mult)
                nc.vector.tensor_tensor(w1[:], w1[:], t2[:], op=OP.mult)
                nc.scalar.activation(w1[:], w1[:], ACTF.Copy, bias=1.0, scale=1.0)
                u = wrk.tile([128, JB], F32, tag=f"u{tag}")
                nc.scalar.activation(u[:], t[:], ACTF.Copy, bias=1.0, scale=-1.0)
                w2 = wrk.tile([128, JB], F32, tag=f"w2{tag}")
                nc.scalar.activation(w2[:], u[:], ACTF.Copy,
                                     bias=-(A + 3.0), scale=A + 2.0)
                u2 = wrk.tile([128, JB], F32, tag=f"u2{tag}")
                nc.vector.tensor_tensor(u2[:], u[:], u[:], op=OP.mult)
                nc.vector.tensor_tensor(w2[:], w2[:], u2[:], op=OP.mult)
                nc.scalar.activation(w2[:], w2[:], ACTF.Copy, bias=1.0, scale=1.0)
                w3 = wrk.tile([128, JB], F32, tag=f"w3{tag}")
                nc.vector.tensor_tensor(w3[:], w0[:], w1[:], op=OP.add)
                nc.vector.tensor_tensor(w3[:], w3[:], w2[:], op=OP.add)
                nc.scalar.activation(w3[:], w3[:], ACTF.Copy, bias=1.0, scale=-1.0)
                return w0, w1, w2, w3

            wx = cubic(tx, "x")
            wy = cubic(ty, "y")

            # pack wx into [128, JB*4] (s-minor), then outer product with wy
            wxp = wbp.tile([128, JB * 4], F32, tag="wxp")
            for s in range(4):
                dst = bass.AP(wxp.tensor, wxp.offset + s, [wxp.ap[0], [4, JB]])
                nc.scalar.copy(dst, wx[s][:])
            wp = wbp.tile([128, JB * 16], F16, tag="wp")
            for r in range(4):
                # wp[.., jj, s, r] = wxp[jj, s] * wy_r[jj]
                dst = bass.AP(wp.tensor, wp.offset + r,
                              [wp.ap[0], [16, JB], [4, 4]])
                src0 = bass.AP(wxp.tensor, wxp.offset,
                               [wxp.ap[0], [4, JB], [1, 4]])
                src1 = bass.AP(wy[r].tensor, wy[r].offset,
                               [wy[r].ap[0], [1, JB], [0, 4]])
                nc.vector.tensor_tensor(dst, src0, src1, op=OP.mult)

            for t in range(NSUB):
                jsub = jb + t * JW
                ybase = jsub + 2                # table row of fy_local = 0
                # idx16 = idxf[:, t*JW:(t+1)*JW] - t*JW*XT  (subtile rebase)
                idxs = wrks.tile([128, JW], F32, tag="idxs")
                islice = bass.AP(idxf.tensor, idxf.offset + t * JW,
                                 [idxf.ap[0], [1, JW]])
                nc.scalar.activation(idxs[:], islice, ACTF.Copy,
                                     bias=float(-t * JW * XT), scale=1.0)
                idx16 = wrks.tile([128, JW], I16, tag="idx16")
                nc.vector.tensor_copy(idx16[:], idxs[:])

                # bounce idx to DRAM, read back wrapped, replicate to 128
                bnc = bncpool.tile([128 * JW], I16, tag="bnc")
                nc.sync.dma_start(
                    bass.AP(bnc.tensor, bnc.offset, [[JW, 128], [1, JW]]),
                    idx16[:])
                # readback in (h, jj)-major order: big contiguous bursts
                idxT = wrks.tile([128, JW * 8], I16, tag="idxT")
                nc.sync.dma_start(
                    bass.AP(idxT.tensor, idxT.offset,
                            [[idxT.ap[0][0], 16], [1, JW * 8]]),
                    bass.AP(bnc.tensor, bnc.offset,
                            [[JW, 16], [16 * JW, 8], [1, JW]]))
                # idxs_ap[q, m] = idx of pixel n = m*16+q ; n = jj*128 + p
                #   -> q = p%16, m = jj*8 + p//16 : reorder (h,jj)->(jj,h)
                idxw = wrks.tile([128, JW * 8], I16, tag="idxw")
                nc.vector.tensor_copy(
                    bass.AP(idxw.tensor, idxw.offset,
                            [[idxw.ap[0][0], 16], [8, JW], [1, 8]]),
                    bass.AP(idxT.tensor, idxT.offset,
                            [[idxT.ap[0][0], 16], [1, JW], [JW, 8]]))
                for k in range(3):
                    p = 16 << k
                    nc.sync.dma_start(
                        bass.AP(idxw.tensor,
                                idxw.offset + p * idxw.ap[0][0],
                                [[idxw.ap[0][0], p], [1, JW * 8]]),
                        bass.AP(idxw.tensor, idxw.offset,
                                [[idxw.ap[0][0], p], [1, JW * 8]]))

                # bulk gather: 8192 patches of 256B, in 2048-desc calls
                # spread over all 4 SWDGE queues to dodge ring backpressure
                G = gp.tile([128, JW, 128], F16, tag="G")
                in_ap = bass.AP(tab.tensor,
                                tab.offset + ybase * XT * 128,
                                [[128, 107 * XT], [1, 128]])
                NI = 128 * JW
                NSP = 2048
                for m in range(NI // NSP):
                    nc.gpsimd.dma_gather(
                        out_ap=G[:, m * (NSP // 128):(m + 1) * (NSP // 128), :],
                        in_ap=in_ap,
                        idxs_ap=idxw[:, m * (NSP // 16):(m + 1) * (NSP // 16)],
                        num_idxs=NSP,
                        num_idxs_reg=NSP,
                        elem_size=128,
                        elem_step=128,
                        single_packet=False,
                        queue_num=qn % 4,
                    )
                    qn += 1

                # expand weights to one per patch element on ACT:
                # wp_exp[jj, s, r, c] = wp[jj, s, r]
                wpx = wxpool.tile([128, JW * 128], F16, tag="wpx")
                nc.scalar.copy(
                    bass.AP(wpx.tensor, wpx.offset,
                            [wpx.ap[0], [128, JW], [32, 4], [8, 4], [1, 8]]),
                    bass.AP(wp.tensor, wp.offset + t * JW * 16,
                            [wp.ap[0], [16, JW], [4, 4], [1, 4], [0, 8]]))

                # combine: P = G * wp_exp — all-contiguous f16 TT (2x mode),
                # written in place over wp_exp
                nfree = JW * 128
                src1 = bass.AP(wpx.tensor, wpx.offset, [wpx.ap[0], [1, nfree]])
                src0 = bass.AP(G.tensor, G.offset, [G.ap[0], [1, nfree]])
                nc.vector.tensor_tensor(src1, src0, src1, op=OP.mult)
                P = wpx

                def halve(buf, npx, stride, n, tag):
                    o = lp.tile([128, JW * stride * (n // 2)], F16, tag=tag)
                    i0 = bass.AP(buf.tensor, buf.offset,
                                 [buf.ap[0], [stride * n, npx],
                                  [stride * 2, n // 2], [1, stride]])
                    i1 = bass.AP(buf.tensor, buf.offset + stride,
                                 [buf.ap[0], [stride * n, npx],
                                  [stride * 2, n // 2], [1, stride]])
                    od = bass.AP(o.tensor, o.offset,
                                 [o.ap[0], [stride * (n // 2), npx],
                                  [stride, n // 2], [1, stride]])
                    nc.vector.tensor_tensor(od, i0, i1, op=OP.add)
                    return o

                L1 = halve(P, JW, 32, 4, "L1")   # sum s pairs
                L2 = halve(L1, JW, 32, 2, "L2")
                L3 = halve(L2, JW, 8, 4, "L3")   # sum r pairs
                of = outp.tile([128, 8 * JW], F32, tag="of")
                i0 = bass.AP(L3.tensor, L3.offset, [L3.ap[0], [16, JW], [1, 8]])
                i1 = bass.AP(L3.tensor, L3.offset + 8, [L3.ap[0], [16, JW], [1, 8]])
                od = bass.AP(of.tensor, of.offset, [of.ap[0], [1, JW], [JW, 8]])
                nc.vector.tensor_tensor(od, i0, i1, op=OP.add)

                # write out[c, IG+p, jsub:jsub+JW]
                dsto = bass.AP(out, IG * W + jsub,
                               [[W, 128], [RPC * W, 8], [1, JW]])
                nc.sync.dma_start(dsto, of[:])


_NC_CACHE = None


def kernel(x: np.ndarray, grid: np.ndarray) -> np.ndarray:
    global _NC_CACHE
    if _NC_CACHE is None:
        _NC_CACHE = build_nc()
    nc = _NC_CACHE

    x0 = np.ascontiguousarray(x[0], dtype=np.float32)        # [C, H, W]
    g0 = np.ascontiguousarray(grid[0], dtype=np.float32)     # [H, W, 2]

    in_maps = []
    for k in range(N_CORES):
        I0 = k * RPC
        xsl = np.zeros((YS + 4, XS, C), dtype=np.float32)
        c0 = I0 - PAD
        lo, hi = max(0, c0), min(W, c0 + XS)
        xsl[PAD:PAD + H, lo - c0:hi - c0, :] = \
            x0[:, :, lo:hi].transpose(1, 2, 0)
        grc = np.ascontiguousarray(g0[I0:I0 + RPC]).copy()
        grc[..., 0] -= I0 / 1024.0   # fold per-core x-base into gx
        in_maps.append({"xs": xsl, "gr": grc})

    res = run_bass_kernel_spmd(nc, in_maps, core_ids=list(range(N_CORES)),
                               trace=False)
    global _LAST_EXEC_NS, _LAST_RES
    _LAST_EXEC_NS = res.exec_time_ns
    _LAST_RES = res
    out = np.empty((1, C, H, W), dtype=np.float32)
    for k in range(N_CORES):
        out[0, :, k * RPC:(k + 1) * RPC, :] = res.results[k]["out"]
    return out


# revision 23
# speedup vs baseline: 2.7502x; 1.0401x over previous
"""Bicubic grid_sample (transpose-like warp) for Trainium2, 8 NeuronCores.

Strategy: shard output rows across cores (256 rows/core). The warp maps
output (i, j) -> input (y ~ j +- 22, x ~ i +- 22), so each core needs an
x-column slab of the image. On device:
  phase 0: convert the f32 slab to an f16 copy in DRAM (contiguous DVE).
  phase 1: repack the f16 slab into a patch table where each 256B token
    holds the full 4x4x8ch bicubic patch at (y0, x0) ([s4 r4 c8] f16).
    Tokens are assembled in SBUF (engine copies do the 16x replication)
    and written with large contiguous DMAs.
  phase 2: per 128x64-pixel subtile: block-level cubic weights / floors /
    indices on DVE+ACT, one 256B-patch dma_gather per pixel (2048-desc
    calls over 4 SWDGE queues), weight-expand on ACT, multiply (DVE 2x
    mode) + tree-reduce, write out.
Phase 1 emission is interleaved with phase 2 so the table build overlaps
gathers of earlier rows (subtile-range deps permit it).
"""
import os, sys, types
sys.path.insert(0, "/opt/trn_rl_repo")
import numpy as np

try:  # register NTFF profile hook so BASS_TRACE=1 can measure HW time
    import antenv
    if "antenv.axon_hooks" not in sys.modules:
        from trn_agent_boot.trn_boot import _ntff_profile_via_ctypes
        _h = _ntff_profile_via_ctypes("/opt/axon/libaxon_pjrt.so")
        _m = types.ModuleType("antenv.axon_hooks")
        _m.get_axon_ntff_profile_hook = lambda: _h
        _m.set_axon_ntff_profile_hook = lambda h: None
        sys.modules["antenv.axon_hooks"] = _m
        antenv.axon_hooks = _m
except Exception:
    pass

import concourse.bass as bass
import concourse.bacc as bacc
import concourse.mybir as mybir
import concourse.tile as tile
from concourse import library_config
from concourse.bass_utils import run_bass_kernel_spmd

F32 = mybir.dt.float32
F16 = mybir.dt.float16
I16 = mybir.dt.int16
I32 = mybir.dt.int32
OP = mybir.AluOpType
ACTF = mybir.ActivationFunctionType

N_CORES = 8
H = W = 2048
C = 8
RPC = H // N_CORES          # output rows per core = 256
PAD = 24                    # y halo rows on each side
YS = H + 2 * PAD            # 2096 slab rows
XS = 308                    # slab cols: [I0-24, I0+284)
XT = 304                    # table x positions (token x0-1 in [0, 304))
XQ = 76                     # table x per staging quarter
TABR = YS + 4               # table rows
JB = 256                    # j-block width (weights/idx per block)
JW = 64                     # j-subtile width
A = -0.75                   # bicubic constant


def build_nc():
    nc = bacc.Bacc("TRN2", target_bir_lowering=False, debug=False,
                   num_devices=N_CORES, num_swdge_queues=4)
    xs = nc.dram_tensor("xs", [YS + 4, XS, C], F32, kind="ExternalInput")
    gr = nc.dram_tensor("gr", [RPC, W, 2], F32, kind="ExternalInput")
    out = nc.dram_tensor("out", [C, RPC, W], F32, kind="ExternalOutput")

    with tile.TileContext(nc) as tc:
        nc.gpsimd.load_library(library_config.mlp)
        import contextlib
        with contextlib.ExitStack() as ctx:
            _build_body(ctx, tc, nc, xs, gr, out)
    nc.compile()
    return nc


def _build_body(ctx, tc, nc, xs, gr, out):
    dram = ctx.enter_context(tc.tile_pool(name="dram", bufs=1, space="DRAM"))
    tab = dram.tile([TABR * XT, 128], F16, tag="tab")
    xsh = dram.tile([(YS + 4) * XS * C], F16, tag="xsh")

    # ---------------- phase 0: f32 slab -> f16 slab in DRAM ----------------
    with tc.tile_pool(name="p0", bufs=2) as p0:
        CH = 128
        n_ch = (YS + 4 + CH - 1) // CH
        for cb in range(n_ch):
            y0 = cb * CH
            rows = min(CH, YS + 4 - y0)
            t = p0.tile([128, XS * C], F32, tag="t")
            nc.sync.dma_start(
                bass.AP(t.tensor, t.offset, [[t.ap[0][0], rows], [1, XS * C]]),
                bass.AP(xs, y0 * XS * C, [[XS * C, rows], [1, XS * C]]))
            f = p0.tile([128, XS * C], F16, tag="f")
            nc.vector.tensor_copy(
                bass.AP(f.tensor, f.offset, [[f.ap[0][0], rows], [1, XS * C]]),
                bass.AP(t.tensor, t.offset, [[t.ap[0][0], rows], [1, XS * C]]))
            nc.sync.dma_start(
                bass.AP(xsh.tensor, xsh.offset + y0 * XS * C,
                        [[XS * C, rows], [1, XS * C]]),
                bass.AP(f.tensor, f.offset, [[f.ap[0][0], rows], [1, XS * C]]))

    # ---------------- phase 1 (emitted lazily, interleaved) ----------------
    ldp = ctx.enter_context(tc.tile_pool(name="ld", bufs=1))
    tgp = ctx.enter_context(tc.tile_pool(name="tg", bufs=1))
    YBK = 128
    n_yb = (YS + YBK - 1) // YBK

    def emit_p1_block(yb):
        y0 = yb * YBK
        rows = min(YBK, YS - y0)
        fs = []
        for r in range(4):
            f = ldp.tile([128, XS * C], F16, tag=f"ld{r}")
            nc.sync.dma_start(
                bass.AP(f.tensor, f.offset, [[f.ap[0][0], rows], [1, XS * C]]),
                bass.AP(xsh.tensor, xsh.offset + (y0 + r) * XS * C,
                        [[XS * C, rows], [1, XS * C]]))
            fs.append(f)
        for q in range(4):
            tg = tgp.tile([128, XQ * 128], F16, tag="tg")
            for r in range(4):
                f = fs[r]
                # tg[y, xq*128 + s*32 + r*8 + c] <- f[y, (76q+xq+s)*8 + c]
                for s in range(4):
                    sdst = bass.AP(tg.tensor, tg.offset + s * 32 + r * 8,
                                   [[tg.ap[0][0], rows], [128, XQ], [1, 8]])
                    ssrc = bass.AP(f.tensor, f.offset + (XQ * q + s) * C,
                                   [[f.ap[0][0], rows], [C, XQ], [1, 8]])
                    if s % 2 == 0:
                        nc.scalar.copy(sdst, ssrc)
                    else:
                        nc.vector.tensor_copy(sdst, ssrc)
            nc.sync.dma_start(
                bass.AP(tab.tensor, tab.offset + y0 * XT * 128 + XQ * q * 128,
                        [[XT * 128, rows], [1, XQ * 128]]),
                bass.AP(tg.tensor, tg.offset,
                        [[tg.ap[0][0], rows], [1, XQ * 128]]))

    p1_state = {"emitted": 0}

    def ensure_rows(y_need):
        while p1_state["emitted"] * YBK < min(y_need, YS):
            emit_p1_block(p1_state["emitted"])
            p1_state["emitted"] += 1

    # ---------------- phase 2 ----------------
    gridp = ctx.enter_context(tc.tile_pool(name="grid", bufs=2))
    wrk = ctx.enter_context(tc.tile_pool(name="wrk", bufs=1))
    wbp = ctx.enter_context(tc.tile_pool(name="wb", bufs=2))
    wrks = ctx.enter_context(tc.tile_pool(name="wrks", bufs=2))
    bncpool = ctx.enter_context(tc.tile_pool(name="bnc", bufs=4, space="DRAM"))
    gp = ctx.enter_context(tc.tile_pool(name="g", bufs=2))
    wxpool = ctx.enter_context(tc.tile_pool(name="wx", bufs=2))
    lp = ctx.enter_context(tc.tile_pool(name="l", bufs=1))
    outp = ctx.enter_context(tc.tile_pool(name="out", bufs=2))

    qn = 0
    for jblk in range(W // JB):
        jb = jblk * JB
        ensure_rows(jb + 302)
        for g in range(2):                # i-halves: rows IG..IG+127
            IG = g * 128
            # grid tile [128 rows, JB cols, 2]
            gt = gridp.tile([128, JB * 2], F32, tag="gt")
            nc.sync.dma_start(
                gt[:],
                bass.AP(gr, IG * W * 2 + jb * 2, [[W * 2, 128], [1, JB * 2]]))
            gx = bass.AP(gt.tensor, gt.offset, [gt.ap[0], [2, JB]])
            gy = bass.AP(gt.tensor, gt.offset + 1, [gt.ap[0], [2, JB]])

            # lx = gx*1024 + 1046.5 (slab x of leftmost tap x0-1)
            # ly = gy*1024 + 1044.5 - jb (block-local y of top tap y0-1)
            lx = wrk.tile([128, JB], F32, tag="lx")
            ly = wrk.tile([128, JB], F32, tag="ly")
            nc.scalar.activation(lx[:], gx, ACTF.Copy, bias=1046.5, scale=1024.0)
            nc.scalar.activation(ly[:], gy, ACTF.Copy,
                                 bias=1044.5 - jb, scale=1024.0)

            def floorpair(v, tag):
                vi = wrk.tile([128, JB], I32, tag=f"vi{tag}")
                nc.vector.tensor_copy(vi[:], v[:])
                vf = wrk.tile([128, JB], F32, tag=f"vf{tag}")
                nc.vector.tensor_copy(vf[:], vi[:])
                co = wrk.tile([128, JB], F32, tag=f"co{tag}")
                nc.vector.tensor_tensor(co[:], vf[:], v[:], op=OP.is_gt)
                nc.vector.tensor_tensor(vf[:], vf[:], co[:], op=OP.subtract)
                fr = wrk.tile([128, JB], F32, tag=f"fr{tag}")
                nc.vector.tensor_tensor(fr[:], v[:], vf[:], op=OP.subtract)
                return vf, fr

            fx, tx = floorpair(lx, "x")   # fx = x0-1 (slab), tx frac
            fy, ty = floorpair(ly, "y")   # fy = y0-1 (block-local), ty frac

            # idxf = fy*XT + fx, then per-subtile rebase and i16 cast
            idxf = wrk.tile([128, JB], F32, tag="idxf")
            nc.vector.scalar_tensor_tensor(idxf[:], fy[:], float(XT), fx[:],
                                           op0=OP.mult, op1=OP.add)
            idxr = wrk.tile([128, JB], F32, tag="idxr")
            for t in range(JB // JW):
                sl_in = bass.AP(idxf.tensor, idxf.offset + t * JW,
                                [idxf.ap[0], [1, JW]])
                sl_out = bass.AP(idxr.tensor, idxr.offset + t * JW,
                                 [idxr.ap[0], [1, JW]])
                nc.scalar.activation(sl_out, sl_in, ACTF.Copy,
                                     bias=float(-t * JW * XT), scale=1.0)
            idx16 = wrks.tile([128, JB], I16, tag="idx16")
            nc.vector.tensor_copy(idx16[:], idxr[:])

            # block-level wrap: bounce p-major, read back (h, jjb)-major,
            # reorder on DVE to (t, jj, h), replicate to 128 partitions
            bnc = bncpool.tile([128 * JB], I16, tag="bnc")
            nc.sync.dma_start(
                bass.AP(bnc.tensor, bnc.offset, [[JB, 128], [1, JB]]),
                idx16[:])
            idxT = wrks.tile([128, JB * 8], I16, tag="idxT")
            nc.sync.dma_start(
                bass.AP(idxT.tensor, idxT.offset,
                        [[idxT.ap[0][0], 16], [1, JB * 8]]),
                bass.AP(bnc.tensor, bnc.offset,
                        [[JB, 16], [16 * JB, 8], [1, JB]]))
            idxw = wrks.tile([128, JB * 8], I16, tag="idxw")
            # idxw[q, t*512 + jj*8 + h] <- idxT[q, h*256 + t*64 + jj]
            nc.vector.tensor_copy(
                bass.AP(idxw.tensor, idxw.offset,
                        [[idxw.ap[0][0], 16], [512, 4], [8, JW], [1, 8]]),
                bass.AP(idxT.tensor, idxT.offset,
                        [[idxT.ap[0][0], 16], [JW, 4], [1, JW], [JB, 8]]))
            for k in range(3):
                p = 16 << k
                nc.sync.dma_start(
                    bass.AP(idxw.tensor, idxw.offset + p * idxw.ap[0][0],
                            [[idxw.ap[0][0], p], [1, JB * 8]]),
                    bass.AP(idxw.tensor, idxw.offset,
                            [[idxw.ap[0][0], p], [1, JB * 8]]))

            # cubic weights for both dirs
            def cubic(t_, tag):
                s0 = wrk.tile([128, JB], F32, tag=f"s0{tag}")
                nc.scalar.activation(s0[:], t_[:], ACTF.Copy, bias=1.0, scale=1.0)
                w0 = wrk.tile([128, JB], F32, tag=f"w0{tag}")
                nc.scalar.activation(w0[:], s0[:], ACTF.Copy,
                                     bias=-5.0 * A, scale=A)
                nc.vector.tensor_tensor(w0[:], w0[:], s0[:], op=OP.mult)
                nc.scalar.activation(w0[:], w0[:], ACTF.Copy, bias=8.0 * A, scale=1.0)
                nc.vector.tensor_tensor(w0[:], w0[:], s0[:], op=OP.mult)
                nc.scalar.activation(w0[:], w0[:], ACTF.Copy, bias=-4.0 * A, scale=1.0)
                w1 = wrk.tile([128, JB], F32, tag=f"w1{tag}")
                nc.scalar.activation(w1[:], t_[:], ACTF.Copy,
                                     bias=-(A + 3.0), scale=A + 2.0)
                t2 = wrk.tile([128, JB], F32, tag=f"t2{tag}")
                nc.vector.tensor_tensor(t2[:], t_[:], t_[:], op=OP.mult)
                nc.vector.tensor_tensor(w1[:], w1[:], t2[:], op=OP.mult)
                nc.scalar.activation(w1[:], w1[:], ACTF.Copy, bias=1.0, scale=1.0)
                u = wrk.tile([128, JB], F32, tag=f"u{tag}")
                nc.scalar.activation(u[:], t_[:], ACTF.Copy, bias=1.0, scale=-1.0)
                w2 = wrk.tile([128, JB], F32, tag=f"w2{tag}")
                nc.scalar.activation(w2[:], u[:], ACTF.Copy,
                                     bias=-(A + 3.0), scale=A + 2.0)
                nc.vector.tensor_tensor(u[:], u[:], u[:], op=OP.mult)
                nc.vector.tensor_tensor(w2[:], w2[:], u[:], op=OP.mult)
                nc.scalar.activation(w2[:], w2[:], ACTF.Copy, bias=1.0, scale=1.0)
                w3 = wrk.tile([128, JB], F32, tag=f"w3{tag}")
                nc.vector.tensor_tensor(w3[:], w0[:], w1[:], op=OP.add)
                nc.vector.tensor_tensor(w3[:], w3[:], w2[:], op=OP.add)
                nc.scalar.activation(w3[:], w3[:], ACTF.Copy, bias=1.0, scale=-1.0)
                return w0, w1, w2, w3

            wx = cubic(tx, "x")
            wy = cubic(ty, "y")

            # pack wx into [128, JB*4] (s-minor), then outer product with wy
            wxp = wbp.tile([128, JB * 4], F32, tag="wxp")
            for s in range(4):
                dst = bass.AP(wxp.tensor, wxp.offset + s, [wxp.ap[0], [4, JB]])
                nc.scalar.copy(dst, wx[s][:])
            wp = wbp.tile([128, JB * 16], F16, tag="wp")
            for r in range(4):
                dst = bass.AP(wp.tensor, wp.offset + r,
                              [wp.ap[0], [16, JB], [4, 4]])
                src0 = bass.AP(wxp.tensor, wxp.offset,
                               [wxp.ap[0], [4, JB], [1, 4]])
                src1 = bass.AP(wy[r].tensor, wy[r].offset,
                               [wy[r].ap[0], [1, JB], [0, 4]])
                nc.vector.tensor_tensor(dst, src0, src1, op=OP.mult)

            for t in range(JB // JW):
                jsub = jb + t * JW
                ybase = jsub + 2                # table row of fy_local = 0
                # bulk gather: 8192 patches of 256B, 2048-desc calls
                G = gp.tile([128, JW, 128], F16, tag="G")
                in_ap = bass.AP(tab.tensor,
                                tab.offset + ybase * XT * 128,
                                [[128, 107 * XT], [1, 128]])
                NI = 128 * JW
                NSP = 2048
                for m in range(NI // NSP):
                    nc.gpsimd.dma_gather(
                        out_ap=G[:, m * (NSP // 128):(m + 1) * (NSP // 128), :],
                        in_ap=in_ap,
                        idxs_ap=idxw[:, t * 512 + m * (NSP // 16):
                                     t * 512 + (m + 1) * (NSP // 16)],
                        num_idxs=NSP,
                        num_idxs_reg=NSP,
                        elem_size=128,
                        elem_step=128,
                        single_packet=False,
                        queue_num=qn % 4,
                    )
                    qn += 1

                # expand weights to one per patch element on ACT
                wpx = wxpool.tile([128, JW * 128], F16, tag="wpx")
                nc.scalar.copy(
                    bass.AP(wpx.tensor, wpx.offset,
                            [wpx.ap[0], [128, JW], [32, 4], [8, 4], [1, 8]]),
                    bass.AP(wp.tensor, wp.offset + t * JW * 16,
                            [wp.ap[0], [16, JW], [4, 4], [1, 4], [0, 8]]))

                # combine: P = G * wpx — all-contiguous f16 TT (2x mode)
                nfree = JW * 128
                src1 = bass.AP(wpx.tensor, wpx.offset, [wpx.ap[0], [1, nfree]])
                src0 = bass.AP(G.tensor, G.offset, [G.ap[0], [1, nfree]])
                nc.vector.tensor_tensor(src1, src0, src1, op=OP.mult)
                P = wpx

                def halve(buf, npx, stride, n, tag):
                    o = lp.tile([128, JW * stride * (n // 2)], F16, tag=tag)
                    i0 = bass.AP(buf.tensor, buf.offset,
                                 [buf.ap[0], [stride * n, npx],
                                  [stride * 2, n // 2], [1, stride]])
                    i1 = bass.AP(buf.tensor, buf.offset + stride,
                                 [buf.ap[0], [stride * n, npx],
                                  [stride * 2, n // 2], [1, stride]])
                    od = bass.AP(o.tensor, o.offset,
                                 [o.ap[0], [stride * (n // 2), npx],
                                  [stride, n // 2], [1, stride]])
                    nc.vector.tensor_tensor(od, i0, i1, op=OP.add)
                    return o

                L1 = halve(P, JW, 32, 4, "L1")   # sum s pairs
                L2 = halve(L1, JW, 32, 2, "L2")
                L3 = halve(L2, JW, 8, 4, "L3")   # sum r pairs
                of = outp.tile([128, 8 * JW], F32, tag="of")
                i0 = bass.AP(L3.tensor, L3.offset, [L3.ap[0], [16, JW], [1, 8]])
                i1 = bass.AP(L3.tensor, L3.offset + 8, [L3.ap[0], [16, JW], [1, 8]])
                od = bass.AP(of.tensor, of.offset, [of.ap[0], [1, JW], [JW, 8]])
                nc.vector.tensor_tensor(od, i0, i1, op=OP.add)

                # write out[c, IG+p, jsub:jsub+JW]
                dsto = bass.AP(out, IG * W + jsub,
                               [[W, 128], [RPC * W, 8], [1, JW]])
                nc.sync.dma_start(dsto, of[:])


_NC_CACHE = None


def kernel(x: np.ndarray, grid: np.ndarray) -> np.ndarray:
    global _NC_CACHE
    if _NC_CACHE is None:
        _NC_CACHE = build_nc()
    nc = _NC_CACHE

    x0 = np.ascontiguousarray(x[0], dtype=np.float32)        # [C, H, W]
    g0 = np.ascontiguousarray(grid[0], dtype=np.float32)     # [H, W, 2]

    in_maps = []
    for k in range(N_CORES):
        I0 = k * RPC
        xsl = np.zeros((YS + 4, XS, C), dtype=np.float32)
        c0 = I0 - PAD
        lo, hi = max(0, c0), min(W, c0 + XS)
        xsl[PAD:PAD + H, lo - c0:hi - c0, :] = \
            x0[:, :, lo:hi].transpose(1, 2, 0)
        grc = np.ascontiguousarray(g0[I0:I0 + RPC]).copy()
        grc[..., 0] -= I0 / 1024.0   # fold per-core x-base into gx
        in_maps.append({"xs": xsl, "gr": grc})

    res = run_bass_kernel_spmd(nc, in_maps, core_ids=list(range(N_CORES)),
                               trace=False)
    global _LAST_EXEC_NS, _LAST_RES
    _LAST_EXEC_NS = res.exec_time_ns
    _LAST_RES = res
    out = np.empty((1, C, H, W), dtype=np.float32)
    for k in range(N_CORES):
        out[0, :, k * RPC:(k + 1) * RPC, :] = res.results[k]["out"]
    return out
